# revision 1
# baseline (speedup 1.0000x reference)
"""AWQ 4-bit quantized linear layer on 8 Trainium2 NeuronCores.

Computes out = x @ W.T + bias where W[o,i] = (q[o,i] - z[o,i//128]) * s[o,i//128],
q/z packed 8x int4 per int32.

Sharding: column-parallel (tensor-parallel on out_features). Each of the 8
cores gets qweight/qzeros/scales/bias rows [c*512, (c+1)*512) and the full
activation (shipped pre-transposed in bf16). Each core dequantizes its weight
shard on-chip — DVE nibble unpack (fused shift+mask tensor_scalar) + scale
multiply with 0-step group-broadcast APs, zero-point subtract on GPSIMD, PE
transposes into [K, N] layout with the PSUM->SBUF copies on the scalar
engine — then runs a bf16 matmul with fp32 PSUM accumulation. The dequant is
emitted in k-chunks interleaved with the first super-block's matmuls so the
(in-order) PE pipeline starts ~15us in; x loads alternate between the two
HWDGE rings. Host concatenates the 8 [B, 512] outputs along the feature axis.
"""

import os
import sys

for _p in ("/opt/trn_rl_repo", "/root/.axon_site/_ro/trn_rl_repo"):
    if os.path.isdir(_p) and _p not in sys.path:
        sys.path.insert(0, _p)

import numpy as np
import ml_dtypes

import concourse.bass as bass
import concourse.tile as tile
from concourse import bacc, mybir
from concourse.masks import make_identity

# Full-problem shapes (hardcoded; harness contract)
B_FULL = 8192
I_FULL = 4096
O_FULL = 4096
N_CORES = 8
GROUP = 128
PACK = 8

BF16 = mybir.dt.bfloat16
F32 = mybir.dt.float32
I32 = mybir.dt.int32


def build_bass(B, I, OS, m_super=512, repeat=1):
    """Build the per-core SPMD program.

    B: batch rows, I: in_features, OS: out_features per core.
    m_super: batch columns processed per super-block (multiple of 128).
    repeat: run the whole body N times (hardware For_i loop) - used only
    for timing measurements (wall-clock slope vs repeat).
    """
    KT = I // 128          # k-tiles (contraction)
    OT = OS // 128         # o-part-tiles in the shard
    NP = I // PACK         # packed int32 words per row
    NG = I // GROUP        # quantization groups
    NGP = (NG + PACK - 1) // PACK
    MSn = B // m_super     # m super-blocks
    M4 = m_super // 128    # 128-row m-tiles per super-block

    nc = bacc.Bacc("TRN2", target_bir_lowering=False)

    xT_d = nc.dram_tensor("xT", [I, B], BF16, kind="ExternalInput")
    qw_d = nc.dram_tensor("qw", [OS, NP], I32, kind="ExternalInput")
    qz_d = nc.dram_tensor("qz", [OS, NGP], I32, kind="ExternalInput")
    sc_d = nc.dram_tensor("sc", [OS, NG], F32, kind="ExternalInput")
    bi_d = nc.dram_tensor("bi", [OS], F32, kind="ExternalInput")
    out_d = nc.dram_tensor("out", [B, OS], F32, kind="ExternalOutput")

    with tile.TileContext(nc) as tc:
        with (
            tc.tile_pool(name="const", bufs=1) as const,
            tc.tile_pool(name="wt", bufs=1) as wtp,
            tc.tile_pool(name="dq", bufs=2) as dq,
            tc.tile_pool(name="xp", bufs=2) as xp,
            tc.tile_pool(name="ob", bufs=4) as ob,
            tc.tile_pool(name="ps", bufs=8, space="PSUM") as ps,
        ):
            rep_ctx = tc.For_i(0, repeat, 1) if repeat > 1 else None
            if rep_ctx is not None:
                rep_ctx.__enter__()
            ident = const.tile([128, 128], BF16)
            make_identity(nc, ident[:])

            # bias broadcast to [128, OS] (varies along free dim of out tiles)
            bias_bc = const.tile([128, OS], F32)
            bi_ap = bi_d[:]
            nc.gpsimd.dma_start(
                out=bias_bc[:],
                in_=bass.AP(tensor=bi_ap.tensor, offset=0, ap=[[0, 128], [1, OS]]),
            )

            # Dequantized weight, [k-tile partition(i), KT, OS] bf16, resident
            WT = wtp.tile([128, KT, OS], BF16)

            # ---- dequantization ----
            # Stage 1: per-o-tile constants (scales, zero*scale, expansions)
            qw_ts, s_fulls, zs_fulls = [], [], []
            for ot in range(OT):
                qw_t = dq.tile([128, NP], I32, name="qw_t", tag="qw_t", bufs=OT)
                nc.sync.dma_start(qw_t[:], qw_d[ot * 128:(ot + 1) * 128, :])
                s_t = dq.tile([128, NG], F32, name="s_t", tag="s_t", bufs=OT)
                nc.sync.dma_start(s_t[:], sc_d[ot * 128:(ot + 1) * 128, :])
                qz_t = dq.tile([128, NGP], I32)
                nc.sync.dma_start(qz_t[:], qz_d[ot * 128:(ot + 1) * 128, :])

                # unpack zero-points: z[o, g], g = 8*pc + j
                z_t = dq.tile([128, NG], I32)
                z_v = z_t.rearrange("p (pc j) -> p pc j", j=PACK)
                for j in range(PACK):
                    nc.vector.tensor_scalar(
                        out=z_v[:, :, j],
                        in0=qz_t[:],
                        scalar1=4 * j,
                        scalar2=0xF,
                        op0=mybir.AluOpType.logical_shift_right,
                        op1=mybir.AluOpType.bitwise_and,
                    )
                # int32 x f32 -> f32 (DVE converts inputs before the ALU)
                zs_t = dq.tile([128, NG], F32, name="zs_t", tag="zs_t", bufs=OT)
                nc.vector.tensor_mul(zs_t[:], z_t[:], s_t[:])
                qw_ts.append(qw_t)
                s_fulls.append(s_t)
                zs_fulls.append(zs_t)

            # xT viewed as [p, kt, b] so one DMA loads all k-tiles of a
            # super-block (amortizes HWDGE fixed cost)
            xT_v = xT_d.rearrange("(kt p) b -> p kt b", p=128)
            out_v = out_d.rearrange("(ms m4 p) o -> ms p m4 o", p=128, m4=M4)

            def load_x(ms):
                xtile = xp.tile([128, KT, m_super], BF16, name="xtile", tag="xtile")
                eng = nc.sync if ms % 2 == 0 else nc.scalar
                eng.dma_start(
                    xtile[:], xT_v[:, :, ms * m_super:(ms + 1) * m_super]
                )
                return xtile

            def mm_run(pss, xtile, m4, ks):
                # consecutive matmuls into the SAME psum bank (avoids
                # per-instruction psum bank cycling)
                for k in ks:
                    nc.tensor.matmul(
                        pss[m4][:],
                        xtile[:, k, m4 * 128:(m4 + 1) * 128],
                        WT[:, k, :],
                        start=(k == 0),
                        stop=(k == KT - 1),
                    )

            def evict(pss, ms):
                o_sb = ob.tile([128, M4, OS], F32, name="o_sb", tag="o_sb")
                for m4 in range(M4):
                    nc.vector.tensor_add(o_sb[:, m4, :], pss[m4][:], bias_bc[:])
                # store via the second HWDGE ring (Activation) to keep the
                # SP ring free for x loads
                nc.scalar.dma_start(out_v[ms], o_sb[:])

            # Stage 2: unpack + scale + transpose in chunks of k-tiles, with
            # the first super-block's matmuls interleaved chunk-by-chunk.
            # PE executes in program order, so transposes must alternate with
            # matmuls in emission order for the pipeline to start early.
            KCH = min(8, KT)           # k-tiles per chunk
            PCH = KCH * 16             # packed words per chunk
            xtile0 = load_x(0)
            pss0 = [ps.tile([128, OS], F32, name="acc", tag="acc")
                    for _ in range(M4)]
            for kc in range((KT + KCH - 1) // KCH):
                for ot in range(OT):
                    psl = slice(kc * PCH, (kc + 1) * PCH)
                    # per-group scale / zero*scale read with a 0-step inner
                    # dim (each group value repeated 16x along the free dim)
                    def bcast(t):
                        sl = t[:, kc * KCH:(kc + 1) * KCH]
                        return bass.AP(tensor=sl.tensor, offset=sl.offset,
                                       ap=[sl.ap[0], sl.ap[1], [0, 16]])
                    s_b = bcast(s_fulls[ot])
                    zs_b = bcast(zs_fulls[ot])
                    # W[o, 8p+j] = nib * s - z*s for p in this chunk
                    W_sb = dq.tile([128, PCH * PACK], BF16, name="W_sb", tag="W_sb")
                    W_v = W_sb.rearrange("p (pk j) -> p pk j", j=PACK)
                    for j in range(PACK):
                        nib = dq.tile([128, PCH], I32)
                        nc.vector.tensor_scalar(
                            out=nib[:],
                            in0=qw_ts[ot][:, psl],
                            scalar1=4 * j,
                            scalar2=0xF,
                            op0=mybir.AluOpType.logical_shift_right,
                            op1=mybir.AluOpType.bitwise_and,
                        )
                        nibf = dq.tile([128, PCH], F32)
                        nc.vector.tensor_tensor(
                            out=nibf.rearrange("p (g r) -> p g r", r=16),
                            in0=nib.rearrange("p (g r) -> p g r", r=16),
                            in1=s_b, op=mybir.AluOpType.mult)
                        nc.gpsimd.tensor_tensor(
                            out=W_v[:, :, j].rearrange("p (g r) -> p g r", r=16),
                            in0=nibf.rearrange("p (g r) -> p g r", r=16),
                            in1=zs_b, op=mybir.AluOpType.subtract)

                    # transpose [128 o, 128 i] blocks -> WT[i, k, o]
                    for kl in range(KCH):
                        k = kc * KCH + kl
                        tp = ps.tile([128, 128], BF16, name="acc", tag="acc")
                        nc.tensor.transpose(
                            tp[:], W_sb[:, kl * 128:(kl + 1) * 128], ident[:]
                        )
                        nc.scalar.copy(WT[:, k, ot * 128:(ot + 1) * 128], tp[:])
                # ms=0 matmuls for this chunk's k-tiles (8 consecutive
                # same-bank matmuls per m4)
                ks = [k for k in range(kc * KCH, min((kc + 1) * KCH, KT))]
                for m4 in range(M4):
                    mm_run(pss0, xtile0, m4, ks)
            evict(pss0, 0)

            # ---- remaining super-blocks ----
            for ms in range(1, MSn):
                xtile = load_x(ms)
                pss = [ps.tile([128, OS], F32, name="acc", tag="acc")
                       for _ in range(M4)]
                for m4 in range(M4):
                    mm_run(pss, xtile, m4, range(KT))
                evict(pss, ms)

            if rep_ctx is not None:
                rep_ctx.__exit__(None, None, None)

    nc.compile()
    return nc


_NC_CACHE = {}


def _get_nc(B, I, OS, repeat=1):
    key = (B, I, OS, repeat)
    if key not in _NC_CACHE:
        _NC_CACHE[key] = build_bass(B, I, OS, repeat=repeat)
    return _NC_CACHE[key]


def make_in_maps(x, qweight, qzeros, scales, bias, n_cores=N_CORES):
    O = qweight.shape[0]
    OS = O // n_cores
    xT = np.ascontiguousarray(x.T).astype(ml_dtypes.bfloat16)
    in_maps = []
    for c in range(n_cores):
        sl = slice(c * OS, (c + 1) * OS)
        in_maps.append({
            "xT": xT,
            "qw": np.ascontiguousarray(qweight[sl]),
            "qz": np.ascontiguousarray(qzeros[sl]),
            "sc": np.ascontiguousarray(scales[sl]),
            "bi": np.ascontiguousarray(bias[sl]),
        })
    return in_maps


def kernel(x, qweight, qzeros, scales, bias):
    from concourse.bass_utils import run_bass_kernel_spmd

    B, I = x.shape
    O = qweight.shape[0]
    OS = O // N_CORES
    nc = _get_nc(B, I, OS)
    in_maps = make_in_maps(x, qweight, qzeros, scales, bias)
    res = run_bass_kernel_spmd(nc, in_maps, core_ids=list(range(N_CORES)))
    out = np.concatenate([res.results[c]["out"] for c in range(N_CORES)], axis=1)
    return out.astype(np.float32)

